# revision 7
# baseline (speedup 1.0000x reference)
"""ClusterSoftmax (topk_masking) distributed Bass kernel for 8 TRN2 NeuronCores.

Reference semantics (for x >= 0, N = 16777216):
    mask  = x != 0
    e     = where(mask, exp(x), 0)
    denom = sum(e)                # over nonzero entries only
    out   = x * e / denom         # == x * exp(x) / denom  (x==0 rows give 0)

Sharding: x split into 8 contiguous shards of 2M elements, one per core,
viewed as [128, 16384] (partition-major). Each core streams variable-size
column tiles: ScalarE computes exp with a free-axis accumulation, VectorE
counts zeros (exp(0)=1 must be backed out of the denom), then ONE scalar
is all-reduced across the 8 cores and the output x*exp(x)/denom is
produced by a single fused scalar_tensor_tensor op per tile. x and exp(x)
stay SBUF-resident, so HBM traffic is 8 MiB in + 8 MiB out per core.
A dependency-free dummy AllReduce issued at kernel start prewarms the
collectives firmware so the real scalar AllReduce runs on a hot path.
"""

import sys

import numpy as np

for _p in ("/root/.axon_site/_ro/trn_rl_repo", "/opt/trn_rl_repo"):
    if _p not in sys.path:
        sys.path.append(_p)

from concourse import bacc, bass, bass_isa, bass_utils, mybir, tile

N = 16777216
NCORES = 8
SHARD = N // NCORES          # 2097152 per core
P = 128                      # SBUF partitions
F = SHARD // P               # 16384 free elems per partition
# ascending tile widths: small lead tiles shorten the pipeline ramp,
# and the last tile's compute tail delays the AllReduce trigger least
# when the widths grow toward the end... (kept moderate at the tail)
TILES = [512, 1024, 1536, 2048, 2048, 2560, 3072, 3584]
assert sum(TILES) == F
NT = len(TILES)

F32 = mybir.dt.float32


def _build():
    nc = bacc.Bacc(
        "TRN2", target_bir_lowering=False, debug=False, num_devices=NCORES
    )
    x_d = nc.dram_tensor("x", [P, F], F32, kind="ExternalInput")
    o_d = nc.dram_tensor("out", [P, F], F32, kind="ExternalOutput")

    with tile.TileContext(nc) as tc:
        with (
            tc.tile_pool(name="xp", bufs=1) as xp,
            tc.tile_pool(name="tp", bufs=1) as tp,
            tc.tile_pool(name="wp", bufs=3) as wp,
            tc.tile_pool(name="mp", bufs=2) as mp,
            tc.tile_pool(name="sp", bufs=1) as sp,
            tc.tile_pool(name="dp", bufs=1, space="DRAM") as dp,
        ):
            # dependency-free dummy collective: wakes the ncfw collectives
            # firmware during phase 1 so the real AllReduce is hot-path
            dwi = dp.tile([1, 1], F32, name="dwi", tag="dwi")
            dwo = dp.tile([1, 1], F32, name="dwo", tag="dwo",
                          addr_space="Shared")
            nc.sync.dma_start(out=dwi[:], in_=x_d.ap()[0:1, 0:1])
            nc.gpsimd.collective_compute(
                "AllReduce", mybir.AluOpType.add,
                replica_groups=[list(range(NCORES))],
                ins=[dwi.opt()], outs=[dwo.opt()],
            )

            # accumulator columns: [0, NT) = per-partition sums of exp(x)
            # over ALL elements; [NT, 2*NT) = per-partition +count(x == 0).
            # Local denom contribution = sum(cols A) - sum(cols B), since
            # each zero contributes exp(0) = 1 to the exp sum.
            acc = sp.tile([P, 2 * NT], F32, name="acc", tag="acc")

            xs, ts = [], []
            c0 = 0
            for i, tf in enumerate(TILES):
                xt = xp.tile([P, tf], F32, name=f"xt{i}", tag=f"xt{i}",
                             bufs=1)
                nc.sync.dma_start(out=xt[:], in_=x_d.ap()[:, c0:c0 + tf])
                tt = tp.tile([P, tf], F32, name=f"tt{i}", tag=f"tt{i}",
                             bufs=1)
                nc.scalar.activation(
                    tt[:], xt[:], mybir.ActivationFunctionType.Exp,
                    accum_out=acc[:, i:i + 1],
                )
                mt = mp.tile([P, tf], F32, name=f"mt{i}", tag="mt")
                # out = (x == 0) as 1.0/0.0; op1 names the accum reduce op
                nc.vector.tensor_scalar(
                    mt[:], xt[:], 0.0, None,
                    mybir.AluOpType.is_equal, mybir.AluOpType.add,
                    accum_out=acc[:, NT + i:NT + i + 1],
                )
                xs.append(xt)
                ts.append(tt)
                c0 += tf

            # local denom contribution: sum_exp - count_zeros, per partition
            ppa = sp.tile([P, 1], F32, name="ppa", tag="ppa")
            nc.vector.tensor_reduce(
                ppa[:], acc[:, :NT], mybir.AxisListType.X, mybir.AluOpType.add
            )
            ppb = sp.tile([P, 1], F32, name="ppb", tag="ppb")
            nc.vector.tensor_reduce(
                ppb[:], acc[:, NT:], mybir.AxisListType.X, mybir.AluOpType.add
            )
            pp = sp.tile([P, 1], F32, name="pp", tag="pp")
            nc.vector.tensor_tensor(
                pp[:], ppa[:], ppb[:], mybir.AluOpType.subtract
            )
            ppr = sp.tile([P, 1], F32, name="ppr", tag="ppr")
            nc.gpsimd.partition_all_reduce(
                ppr[:], pp[:], P, bass_isa.ReduceOp.add
            )

            # one-scalar AllReduce across the 8 cores (DRAM bounce buffers)
            cin = dp.tile([1, 1], F32, name="cin", tag="cin")
            cout = dp.tile([1, 1], F32, name="cout", tag="cout",
                           addr_space="Shared")
            nc.sync.dma_start(out=cin[:], in_=ppr[0:1, :])
            nc.gpsimd.collective_compute(
                "AllReduce", mybir.AluOpType.add,
                replica_groups=[list(range(NCORES))],
                ins=[cin.opt()], outs=[cout.opt()],
            )
            dsb = sp.tile([1, 1], F32, name="dsb", tag="dsb")
            nc.sync.dma_start(out=dsb[:], in_=cout[:])
            dbc = sp.tile([P, 1], F32, name="dbc", tag="dbc")
            nc.gpsimd.partition_broadcast(dbc[:], dsb[:])
            rsb = sp.tile([P, 1], F32, name="rsb", tag="rsb")
            nc.vector.reciprocal(rsb[:], dbc[:])

            # finish: out = (x * (1/denom)) * exp(x), one fused DVE op/tile
            c0 = 0
            for i, tf in enumerate(TILES):
                yt = wp.tile([P, tf], F32, name=f"yt{i}", tag="yt")
                nc.vector.scalar_tensor_tensor(
                    yt[:], xs[i][:], rsb[:], ts[i][:],
                    mybir.AluOpType.mult, mybir.AluOpType.mult,
                )
                nc.sync.dma_start(out=o_d.ap()[:, c0:c0 + tf], in_=yt[:])
                c0 += tf

    nc.compile()
    return nc


_NC_CACHE = None


def _get_nc():
    global _NC_CACHE
    if _NC_CACHE is None:
        _NC_CACHE = _build()
    return _NC_CACHE


def kernel(x: np.ndarray) -> np.ndarray:
    assert x.shape == (N,) and x.dtype == np.float32
    nc = _get_nc()
    shards = np.ascontiguousarray(x).reshape(NCORES, P, F)
    in_maps = [{"x": np.ascontiguousarray(shards[i])} for i in range(NCORES)]
    res = bass_utils.run_bass_kernel_spmd(
        nc, in_maps, core_ids=list(range(NCORES))
    )
    out = np.empty((NCORES, P, F), dtype=np.float32)
    for i in range(NCORES):
        out[i] = res.results[i]["out"]
    return out.reshape(N)


# revision 12
# speedup vs baseline: 1.2298x; 1.2298x over previous
"""ClusterSoftmax (topk_masking) distributed Bass kernel for 8 TRN2 NeuronCores.

Reference semantics (for x >= 0, N = 16777216):
    mask  = x != 0
    e     = where(mask, exp(x), 0)
    denom = sum(e)                # over nonzero entries only
    out   = x * e / denom         # == x * exp(x) / denom  (x==0 rows give 0)

Sharding: x split into 8 contiguous shards of 2M elements, one per core,
viewed as [128, 16384] (partition-major). Each core streams variable-size
column tiles: ScalarE computes exp with a free-axis accumulation, VectorE
counts zeros (exp(0)=1 must be backed out of the denom), then ONE scalar
is all-reduced across the 8 cores and the output x*exp(x)/denom is
produced by a single fused scalar_tensor_tensor op per tile. x and exp(x)
stay SBUF-resident, so HBM traffic is 8 MiB in + 8 MiB out per core.
A dependency-free dummy AllReduce issued at kernel start prewarms the
collectives firmware so the real scalar AllReduce runs on a hot path.
"""

import sys

import numpy as np

for _p in ("/root/.axon_site/_ro/trn_rl_repo", "/opt/trn_rl_repo"):
    if _p not in sys.path:
        sys.path.append(_p)

from concourse import bacc, bass, bass_isa, bass_utils, mybir, tile

N = 16777216
NCORES = 8
SHARD = N // NCORES          # 2097152 per core
P = 128                      # SBUF partitions
F = SHARD // P               # 16384 free elems per partition
TILES = [2048] * 8
assert sum(TILES) == F
NT = len(TILES)

F32 = mybir.dt.float32


def _build():
    nc = bacc.Bacc(
        "TRN2", target_bir_lowering=False, debug=False, num_devices=NCORES
    )
    x_d = nc.dram_tensor("x", [P, F], F32, kind="ExternalInput")
    o_d = nc.dram_tensor("out", [P, F], F32, kind="ExternalOutput")

    with tile.TileContext(nc) as tc:
        with (
            tc.tile_pool(name="xp", bufs=1) as xp,
            tc.tile_pool(name="tp", bufs=1) as tp,
            tc.tile_pool(name="wp", bufs=3) as wp,
            tc.tile_pool(name="mp", bufs=2) as mp,
            tc.tile_pool(name="sp", bufs=1) as sp,
            tc.tile_pool(name="dp", bufs=1, space="DRAM") as dp,
        ):
            # accumulator columns: [0, NT) = per-partition sums of exp(x)
            # over ALL elements; [NT, 2*NT) = per-partition +count(x == 0).
            # Local denom contribution = sum(cols A) - sum(cols B), since
            # each zero contributes exp(0) = 1 to the exp sum.
            acc = sp.tile([P, 2 * NT], F32, name="acc", tag="acc")

            xs, ts = [], []
            c0 = 0
            for i, tf in enumerate(TILES):
                xt = xp.tile([P, tf], F32, name=f"xt{i}", tag=f"xt{i}",
                             bufs=1)
                nc.sync.dma_start(out=xt[:], in_=x_d.ap()[:, c0:c0 + tf])
                tt = tp.tile([P, tf], F32, name=f"tt{i}", tag=f"tt{i}",
                             bufs=1)
                nc.scalar.activation(
                    tt[:], xt[:], mybir.ActivationFunctionType.Exp,
                    accum_out=acc[:, i:i + 1],
                )
                mt = mp.tile([P, tf], F32, name=f"mt{i}", tag="mt")
                # out = (x == 0) as 1.0/0.0; op1 names the accum reduce op
                nc.vector.tensor_scalar(
                    mt[:], xt[:], 0.0, None,
                    mybir.AluOpType.is_equal, mybir.AluOpType.add,
                    accum_out=acc[:, NT + i:NT + i + 1],
                )
                xs.append(xt)
                ts.append(tt)
                c0 += tf

            # local denom contribution: sum_exp - count_zeros, per partition
            ppa = sp.tile([P, 1], F32, name="ppa", tag="ppa")
            nc.vector.tensor_reduce(
                ppa[:], acc[:, :NT], mybir.AxisListType.X, mybir.AluOpType.add
            )
            ppb = sp.tile([P, 1], F32, name="ppb", tag="ppb")
            nc.vector.tensor_reduce(
                ppb[:], acc[:, NT:], mybir.AxisListType.X, mybir.AluOpType.add
            )
            pp = sp.tile([P, 1], F32, name="pp", tag="pp")
            nc.vector.tensor_tensor(
                pp[:], ppa[:], ppb[:], mybir.AluOpType.subtract
            )
            ppr = sp.tile([P, 1], F32, name="ppr", tag="ppr")
            nc.gpsimd.partition_all_reduce(
                ppr[:], pp[:], P, bass_isa.ReduceOp.add
            )

            # one-scalar-per-rank AllGather across the 8 cores (cheaper ncfw
            # path than AllReduce); each core then sums the 8 values locally
            cin = dp.tile([1, 1], F32, name="cin", tag="cin")
            cout = dp.tile([1, NCORES], F32, name="cout", tag="cout",
                           addr_space="Shared")
            nc.sync.dma_start(out=cin[:], in_=ppr[0:1, :])
            nc.gpsimd.collective_compute(
                "AllGather", mybir.AluOpType.bypass,
                replica_groups=[list(range(NCORES))],
                ins=[cin.opt()], outs=[cout.opt()],
            )
            gsb = sp.tile([1, NCORES], F32, name="gsb", tag="gsb")
            nc.sync.dma_start(out=gsb[:], in_=cout[:])
            dsb = sp.tile([1, 1], F32, name="dsb", tag="dsb")
            nc.vector.tensor_reduce(
                dsb[:], gsb[:], mybir.AxisListType.X, mybir.AluOpType.add
            )
            dbc = sp.tile([P, 1], F32, name="dbc", tag="dbc")
            nc.gpsimd.partition_broadcast(dbc[:], dsb[:])
            rsb = sp.tile([P, 1], F32, name="rsb", tag="rsb")
            nc.vector.reciprocal(rsb[:], dbc[:])

            # finish: out = (x * (1/denom)) * exp(x), one fused DVE op/tile
            c0 = 0
            for i, tf in enumerate(TILES):
                yt = wp.tile([P, tf], F32, name=f"yt{i}", tag="yt")
                nc.vector.scalar_tensor_tensor(
                    yt[:], xs[i][:], rsb[:], ts[i][:],
                    mybir.AluOpType.mult, mybir.AluOpType.mult,
                )
                nc.sync.dma_start(out=o_d.ap()[:, c0:c0 + tf], in_=yt[:])
                c0 += tf

    nc.compile()
    return nc


_NC_CACHE = None


def _get_nc():
    global _NC_CACHE
    if _NC_CACHE is None:
        _NC_CACHE = _build()
    return _NC_CACHE


def kernel(x: np.ndarray) -> np.ndarray:
    assert x.shape == (N,) and x.dtype == np.float32
    nc = _get_nc()
    shards = np.ascontiguousarray(x).reshape(NCORES, P, F)
    in_maps = [{"x": np.ascontiguousarray(shards[i])} for i in range(NCORES)]
    res = bass_utils.run_bass_kernel_spmd(
        nc, in_maps, core_ids=list(range(NCORES))
    )
    out = np.empty((NCORES, P, F), dtype=np.float32)
    for i in range(NCORES):
        out[i] = res.results[i]["out"]
    return out.reshape(N)
